# revision 1
# baseline (speedup 1.0000x reference)
"""CBOW negative-sampling loss on 8 Trainium2 NeuronCores.

Strategy (from sharding hint): replicate the embedding tables, data-parallel
over the batch dim. Each core handles 2048 of the 16384 batch rows.

Host side: u_emb and w_emb are concatenated into one [2V, D] bf16 table so
each group needs a single indirect-DMA gather (w-indices offset by +V); bf16
halves both the HBM gather traffic and the DVE element costs.

Per-core kernel layout:
  - batch row b -> chunk c = b // 128, partition p = b % 128.
  - 16 chunks in groups sized (2,4,5,4,1): ramped so the first gather lands
    early and the last group leaves only a short compute tail. Per group ONE
    indirect gather pulls, per partition, n_chunks x (8 u-rows + 6 w-rows) x
    128 bf16.
  - h = sum of the 8 context embeddings: contiguous binary add-tree over all
    chunks of the group at once (3 DVE instructions).
  - dots: one broadcast-mult [P,n,6,128] (bf16) + one X-reduce -> f32 scores.
  - per group: sign pattern [+1,-1,...] then Exp(-x), Ln(x+1) with accum_out
    -> column g of lp [128, n_groups]: sum of softplus(-x) terms.
  - finale: reduce lp rows, PE ones-matmul to collapse partitions -> [1,1]
    (single-descriptor output DMA), host sums the 8 per-core scalars.

loss = sum_b softplus(-score_b) + sum_{b,k} softplus(+neg_score_bk)
"""

import sys

import numpy as np

sys.path.insert(0, "/opt/trn_rl_repo")

import ml_dtypes  # noqa: E402

from concourse import bacc, bass, mybir, tile  # noqa: E402
from concourse.bass_utils import run_bass_kernel_spmd  # noqa: E402

V, D = 100000, 128
B, C, K = 16384, 8, 5
N_CORES = 8
P = 128
B_LOC = B // N_CORES            # 2048 batch rows per core
N_CHUNK = B_LOC // P            # 16 chunks of 128 rows
GROUPS = (1, 2, 4, 5, 4)        # chunks per indirect-DMA gather group
assert sum(GROUPS) == N_CHUNK
J = 1 + K                       # 6 w-rows per batch row (pos + negs)
R = C + J                       # 14 gathered rows per batch row

_NC_CACHE = {}


def _build_bass():
    nc = bacc.Bacc(
        "TRN2",
        target_bir_lowering=False,
        debug=False,
        dynamic_dma_scratch_size=65536,
    )

    bf16 = mybir.dt.bfloat16
    fp32 = mybir.dt.float32
    X = mybir.AxisListType.X
    ADD = mybir.AluOpType.add
    NG = len(GROUPS)

    emb = nc.dram_tensor("emb_cat", [2 * V, D], bf16, kind="ExternalInput")
    gidx = nc.dram_tensor(
        "gidx", [P, N_CHUNK * R], mybir.dt.int32, kind="ExternalInput"
    )
    loss = nc.dram_tensor("loss_part", [1, 1], fp32, kind="ExternalOutput")

    with tile.TileContext(nc) as tc:
        with (
            tc.tile_pool(name="idx", bufs=1) as idx_pool,
            tc.tile_pool(name="gb", bufs=5) as gb_pool,
            tc.tile_pool(name="m", bufs=3) as m_pool,
            tc.tile_pool(name="sc", bufs=2) as sc_pool,
            tc.tile_pool(name="fin", bufs=1) as fin_pool,
            tc.tile_pool(name="ps", bufs=1, space="PSUM") as ps_pool,
        ):
            ones = fin_pool.tile([P, 1], fp32, tag="ones")
            nc.gpsimd.memset(ones[:], 1.0)
            # exp(-x) for all score cols, filled per group; single Ln at end
            ex_all = fin_pool.tile([P, N_CHUNK * J], fp32, tag="ex_all")

            starts = [sum(GROUPS[:g]) for g in range(NG)]
            gb_t = {}

            ix_all = idx_pool.tile([P, N_CHUNK * R], mybir.dt.int32, tag="ix")
            nc.sync.dma_start(out=ix_all[:], in_=gidx[:])

            def issue_gather(g):
                n = GROUPS[g]
                c0 = starts[g]
                gb = gb_pool.tile([P, n * R * D], bf16, tag="gb")
                nc.gpsimd.indirect_dma_start(
                    out=gb[:],
                    out_offset=None,
                    in_=emb[:],
                    in_offset=bass.IndirectOffsetOnAxis(
                        ap=ix_all[:, c0 * R : (c0 + n) * R], axis=0
                    ),
                )
                gb_t[g] = gb

            issue_gather(0)
            for g in range(NG):
                if g + 1 < NG:
                    issue_gather(g + 1)
                n = GROUPS[g]
                gb = gb_t.pop(g)
                g3 = gb[:].rearrange("p (c e) -> p c e", c=n)  # e = R*D

                # h = sum of the 8 context embeddings (cols 0 : 8D of each
                # chunk block); contiguous binary add-tree, all chunks at
                # once, all on DVE (GpSimd has no bf16 speedup and stalls
                # the chain).
                nc.vector.tensor_add(
                    out=g3[:, :, 0 : 4 * D],
                    in0=g3[:, :, 0 : 4 * D],
                    in1=g3[:, :, 4 * D : 8 * D],
                )
                nc.vector.tensor_add(
                    out=g3[:, :, 0 : 2 * D],
                    in0=g3[:, :, 0 : 2 * D],
                    in1=g3[:, :, 2 * D : 4 * D],
                )
                nc.vector.tensor_add(
                    out=g3[:, :, 0:D],
                    in0=g3[:, :, 0:D],
                    in1=g3[:, :, D : 2 * D],
                )
                h4 = g3[:, :, 0:D]  # [P, n, D]

                # m[p, c, j, d] = w[p, c, j, d] * h[p, c, d]
                w4 = g3[:, :, C * D : R * D].rearrange("p c (j d) -> p c j d", j=J)
                m = m_pool.tile([P, n * J * D], bf16, tag="m")
                m4 = m[:].rearrange("p (c j d) -> p c j d", c=n, j=J)
                nc.vector.tensor_mul(
                    out=m4,
                    in0=w4,
                    in1=h4[:, :, None, :].broadcast_to([P, n, J, D]),
                )
                # pre-fold the innermost 128 -> 16 with bf16 adds (~0.3ns/elem)
                # before the TensorReduce (~1.1ns/elem)
                for w_ in (64, 32, 16):
                    nc.vector.tensor_add(
                        out=m4[:, :, :, 0:w_],
                        in0=m4[:, :, :, 0:w_],
                        in1=m4[:, :, :, w_ : 2 * w_],
                    )
                # raw dots (f32): x = [+pos, -negs], neg sign via the reduce
                sc = sc_pool.tile([P, n * J], fp32, tag="sc")
                sc3 = sc[:].rearrange("p (c j) -> p c j", j=J)
                nc.vector.tensor_reduce(
                    out=sc3[:, :, 0:1], in_=m4[:, :, 0:1, 0:16], axis=X, op=ADD
                )
                nc.vector.tensor_reduce(
                    out=sc3[:, :, 1:J],
                    in_=m4[:, :, 1:J, 0:16],
                    axis=X,
                    op=ADD,
                    negate=True,
                )
                # softplus(-x) = ln(1 + exp(-x)); Exp batched per group (one
                # ACT table), Ln once at end.
                c0 = starts[g]
                nc.scalar.activation(
                    out=ex_all[:, c0 * J : (c0 + n) * J],
                    in_=sc[:],
                    func=mybir.ActivationFunctionType.Exp,
                    scale=-1.0,
                )

            # ln(1 + ex) summed over all 96 cols -> per-partition loss [P,1]
            sp = fin_pool.tile([P, N_CHUNK * J], fp32, tag="sp")
            lp1 = fin_pool.tile([P, 1], fp32, tag="lp1")
            nc.scalar.activation(
                out=sp[:],
                in_=ex_all[:],
                func=mybir.ActivationFunctionType.Ln,
                bias=1.0,
                accum_out=lp1[:],
            )
            # collapse partitions via ones-matmul -> [1,1]
            acc = ps_pool.tile([1, 1], fp32, space="PSUM")
            nc.tensor.matmul(out=acc[:], lhsT=ones[:], rhs=lp1[:], start=True, stop=True)
            out_sb = fin_pool.tile([1, 1], fp32, tag="out")
            nc.vector.tensor_copy(out=out_sb[:], in_=acc[:])
            nc.sync.dma_start(out=loss[:], in_=out_sb[:])

    nc.compile()
    return nc


def _get_nc():
    if "nc" not in _NC_CACHE:
        _NC_CACHE["nc"] = _build_bass()
    return _NC_CACHE["nc"]


def _make_in_maps(pos_u, pos_w, neg_w, u_emb, w_emb):
    pos_u = np.asarray(pos_u).astype(np.int32)
    pos_w = np.asarray(pos_w).astype(np.int32)
    neg_w = np.asarray(neg_w).astype(np.int32)
    u_emb = np.asarray(u_emb, dtype=np.float32)
    w_emb = np.asarray(w_emb, dtype=np.float32)

    emb_cat = np.ascontiguousarray(
        np.concatenate([u_emb, w_emb], axis=0).astype(ml_dtypes.bfloat16)
    )

    in_maps = []
    for i in range(N_CORES):
        sl = slice(i * B_LOC, (i + 1) * B_LOC)
        # per batch row: [8 ctx u-idx | pos_w + V | neg_w + V]  -> R = 14
        rows = np.concatenate(
            [pos_u[sl], pos_w[sl, None] + V, neg_w[sl] + V], axis=1
        )  # [B_LOC, 14]
        # batch row b -> (chunk c = b // 128, partition p = b % 128)
        gidx = rows.reshape(N_CHUNK, P, R).transpose(1, 0, 2).reshape(P, N_CHUNK * R)
        in_maps.append(
            {
                "emb_cat": emb_cat,
                "gidx": np.ascontiguousarray(gidx),
            }
        )
    return in_maps


def _install_axon_profile_shim():
    """Provide antenv.axon_hooks (missing in this image) so trace=True can
    capture NTFF profiles via the axon PJRT .so, and keep trace artifacts
    local instead of uploading to a bucket."""
    import contextlib
    import ctypes
    import types

    import concourse.bass_utils as bu

    bu.upload_artifacts = lambda tmpdir: tmpdir

    try:
        from antenv.axon_hooks import get_axon_ntff_profile_hook  # noqa: F401

        return
    except ImportError:
        pass

    mod = types.ModuleType("antenv.axon_hooks")
    holder = {}
    mod.set_axon_ntff_profile_hook = lambda h: holder.__setitem__("h", h)
    mod.get_axon_ntff_profile_hook = lambda: holder.get("h")
    sys.modules["antenv.axon_hooks"] = mod
    import antenv

    antenv.axon_hooks = mod

    so_path = "/opt/axon/libaxon_pjrt.so"
    lib = ctypes.CDLL(so_path)
    if not hasattr(lib, "axon_start_nrt_profile"):
        return
    lib.axon_start_nrt_profile.argtypes = [
        ctypes.POINTER(ctypes.c_int64),
        ctypes.c_size_t,
    ]
    lib.axon_start_nrt_profile.restype = ctypes.c_int64
    lib.axon_stop_nrt_profile.argtypes = [ctypes.c_char_p]
    lib.axon_stop_nrt_profile.restype = ctypes.c_int64

    @contextlib.contextmanager
    def _hook(output_dir, device_ids):
        import jax

        jax.devices()
        if device_ids:
            ids = (ctypes.c_int64 * len(device_ids))(*device_ids)
            rc = lib.axon_start_nrt_profile(ids, len(device_ids))
        else:
            rc = lib.axon_start_nrt_profile(None, 0)
        if rc != 0:
            raise RuntimeError(f"axon_start_nrt_profile rc={rc}")
        try:
            yield
        finally:
            n = lib.axon_stop_nrt_profile(str(output_dir).encode())
            print(f"profile: {n} file(s) written to {output_dir}")

    mod.set_axon_ntff_profile_hook(_hook)


def _run(in_maps, trace=False):
    if trace:
        _install_axon_profile_shim()
    nc = _get_nc()
    return run_bass_kernel_spmd(nc, in_maps, list(range(N_CORES)), trace=trace)


def kernel(pos_u, pos_w, neg_w, u_emb, w_emb):
    in_maps = _make_in_maps(pos_u, pos_w, neg_w, u_emb, w_emb)
    bkr = _run(in_maps, trace=False)
    total = 0.0
    for r in bkr.results:
        total += float(r["loss_part"].astype(np.float64).sum())
    return np.float32(total)


def kernel_traced(pos_u, pos_w, neg_w, u_emb, w_emb):
    """Like kernel() but returns (loss, BassKernelResults) with HW profile."""
    in_maps = _make_in_maps(pos_u, pos_w, neg_w, u_emb, w_emb)
    bkr = _run(in_maps, trace=True)
    total = 0.0
    for r in bkr.results:
        total += float(r["loss_part"].astype(np.float64).sum())
    return np.float32(total), bkr



# revision 6
# speedup vs baseline: 1.1052x; 1.1052x over previous
"""CBOW negative-sampling loss on 8 Trainium2 NeuronCores.

Strategy: replicate the embedding tables, data-parallel over the batch dim.
Each core handles 2048 of the 16384 batch rows.

Host side: u_emb (x1024) and w_emb (x32) are concatenated into one [2V, D]
fp8-e4m3 table so each group needs a single indirect-DMA gather (w-indices
offset by +V). fp8 halves the HBM gather traffic vs bf16; the SWDGE casts
to bf16 on the fly during the gather so DVE keeps its 2x 16-bit rate. The
scale factors keep fp8 values in the normal range and are divided back out
in the activation's scale factor.

Per-core kernel layout:
  - batch row b -> chunk c = b // 128, partition p = b % 128.
  - 16 chunks in ramped groups; per group ONE indirect gather pulls, per
    partition, n_chunks x (8 u-rows + 6 w-rows) x 128 fp8 -> bf16. The
    group's u-rows are ordered k-major (all k=0 rows, then k=1, ...) so
    each k-slab is one contiguous [P, n*D] block.
  - h = sum of the 8 context embeddings: 8 accumulating identity-weight
    matmuls per group (rhs = the k-th slab) on the (otherwise idle)
    TensorEngine -> PSUM, then one ACT copy PSUM -> SBUF bf16. Frees
    ~7us of DVE adds.
  - dots on DVE: one broadcast-mult [P,n,6,128] (bf16, in-place over the
    w block), 3 contiguous fold-adds 128->16, TensorReduce -> f32 scores
    (pos negated so every loss term is softplus(+x)).
  - softplus per group on ACT: Exp then Ln(1+x) with accum_out -> lp[:, g].
    Both functions come from one activation table (id 6) preloaded at
    kernel start, so no mid-kernel table swaps.
  - finale: reduce lp rows, PE ones-matmul to collapse partitions -> [1,1],
    host sums the 8 per-core scalars.

loss = sum_b softplus(-score_b) + sum_{b,k} softplus(+neg_score_bk)
"""

import sys

import numpy as np

sys.path.insert(0, "/opt/trn_rl_repo")

from concourse import bacc, bass, mybir, tile  # noqa: E402
from concourse.bass_utils import run_bass_kernel_spmd  # noqa: E402
from concourse.masks import make_identity  # noqa: E402

V, D = 100000, 128
B, C, K = 16384, 8, 5
N_CORES = 8
P = 128
B_LOC = B // N_CORES            # 2048 batch rows per core
N_CHUNK = B_LOC // P            # 16 chunks of 128 rows
GROUPS = (2, 4, 4, 4, 2)        # chunks per indirect-DMA gather group
assert sum(GROUPS) == N_CHUNK
# matmul moving operand and a PSUM bank both cap at 512 f32 per partition
assert max(GROUPS) * D <= 512
J = 1 + K                       # 6 w-rows per batch row (pos + negs)
R = C + J                       # 14 gathered rows per batch row

S_U = 1024.0                    # host-side scale into fp8 normal range
S_W = 32.0
ACT_SCALE = 1.0 / (S_U * S_W)   # divided back out inside the Exp

_NC_CACHE = {}


def _build_bass():
    nc = bacc.Bacc(
        "TRN2",
        target_bir_lowering=False,
        debug=False,
        dynamic_dma_scratch_size=65536,
    )

    bf16 = mybir.dt.bfloat16
    fp32 = mybir.dt.float32
    fp8 = mybir.dt.float8e4
    X = mybir.AxisListType.X
    ADD = mybir.AluOpType.add
    NG = len(GROUPS)
    EXP = mybir.ActivationFunctionType.Exp
    LN = mybir.ActivationFunctionType.Ln
    COPY = mybir.ActivationFunctionType.Copy

    emb = nc.dram_tensor("emb_cat", [2 * V, D], fp8, kind="ExternalInput")
    gidx = nc.dram_tensor(
        "gidx", [P, N_CHUNK * R], mybir.dt.int32, kind="ExternalInput"
    )
    loss = nc.dram_tensor("loss_part", [1, 1], fp32, kind="ExternalOutput")

    with tile.TileContext(nc) as tc:
        with (
            tc.tile_pool(name="idx", bufs=1) as idx_pool,
            tc.tile_pool(name="gb", bufs=3) as gb_pool,
            tc.tile_pool(name="sc", bufs=2) as sc_pool,
            tc.tile_pool(name="sp", bufs=2) as sp_pool,
            tc.tile_pool(name="fin", bufs=1) as fin_pool,
            tc.tile_pool(name="hp", bufs=2, space="PSUM") as hp_pool,
            tc.tile_pool(name="ps", bufs=1, space="PSUM") as ps_pool,
        ):
            # one table (id 6: natural_log_exp_and_others) serves Exp AND Ln
            nc.scalar.add_instruction(mybir.InstLoadActFuncSet(
                name=nc.get_next_instruction_name(), act_func_set_id=6,
                ins=[], outs=[]))

            ix_all = idx_pool.tile([P, N_CHUNK * R], mybir.dt.int32, tag="ix")
            nc.sync.dma_start(out=ix_all[:], in_=gidx[:])

            ident = fin_pool.tile([P, P], bf16, tag="ident")
            make_identity(nc, ident[:])
            ones = fin_pool.tile([P, 1], fp32, tag="ones")
            nc.gpsimd.memset(ones[:], 1.0)

            h_sb = fin_pool.tile([P, N_CHUNK * D], bf16, tag="h_sb")
            h16 = h_sb[:].rearrange("p (c d) -> p c d", c=N_CHUNK)
            lp = fin_pool.tile([P, NG], fp32, tag="lp")

            starts = [sum(GROUPS[:g]) for g in range(NG)]
            gb_t = {}

            def issue_gather(g):
                n = GROUPS[g]
                c0 = starts[g]
                gb = gb_pool.tile([P, n * R * D], bf16, tag="gb")
                nc.gpsimd.indirect_dma_start(
                    out=gb[:],
                    out_offset=None,
                    in_=emb[:],
                    in_offset=bass.IndirectOffsetOnAxis(
                        ap=ix_all[:, c0 * R : (c0 + n) * R], axis=0
                    ),
                )
                gb_t[g] = gb

            issue_gather(0)
            for g in range(NG):
                if g + 1 < NG:
                    issue_gather(g + 1)
                n = GROUPS[g]
                c0 = starts[g]
                gb = gb_t.pop(g)

                # h = sum of the 8 context embeddings, on the TensorEngine:
                # 8 accumulating identity matmuls (rhs = the k-th u-slab,
                # contiguous [P, n*D]) -> PSUM [P, n*D] f32.
                hp = hp_pool.tile([P, n * D], fp32, tag="hp")
                for k in range(C):
                    nc.tensor.matmul(
                        out=hp[:],
                        lhsT=ident[:],
                        rhs=gb[:, k * n * D : (k + 1) * n * D],
                        start=(k == 0),
                        stop=(k == C - 1),
                    )
                # PSUM f32 -> SBUF bf16 on the Scalar engine (idle otherwise)
                nc.scalar.activation(
                    out=h_sb[:, c0 * D : (c0 + n) * D], in_=hp[:], func=COPY
                )

                # m[p, c, j, d] = w[p, c, j, d] * h[p, c, d], in place over w
                w4 = gb[:, C * n * D : R * n * D].rearrange(
                    "p (c j d) -> p c j d", c=n, j=J
                )
                h4 = h16[:, c0 : c0 + n]
                nc.vector.tensor_mul(
                    out=w4,
                    in0=w4,
                    in1=h4[:, :, None, :].broadcast_to([P, n, J, D]),
                )
                # fold the innermost 128 -> 16 with bf16 adds before the
                # (1x-rate) TensorReduce
                for w_ in (64, 32, 16):
                    nc.vector.tensor_add(
                        out=w4[:, :, :, 0:w_],
                        in0=w4[:, :, :, 0:w_],
                        in1=w4[:, :, :, w_ : 2 * w_],
                    )
                # raw dots (f32), sign-flipped so every term is softplus(+x)
                sc = sc_pool.tile([P, n * J], fp32, tag="sc")
                sc3 = sc[:].rearrange("p (c j) -> p c j", j=J)
                nc.vector.tensor_reduce(
                    out=sc3[:, :, 0:1], in_=w4[:, :, 0:1, 0:16], axis=X, op=ADD,
                    negate=True,
                )
                nc.vector.tensor_reduce(
                    out=sc3[:, :, 1:J], in_=w4[:, :, 1:J, 0:16], axis=X, op=ADD,
                )
                # softplus(x) = ln(1 + exp(x)); accumulate into lp[:, g]
                sp = sp_pool.tile([P, n * J], fp32, tag="sp")
                nc.scalar.activation(out=sp[:], in_=sc[:], func=EXP,
                                     scale=ACT_SCALE)
                nc.scalar.activation(out=sp[:], in_=sp[:], func=LN, bias=1.0,
                                     accum_out=lp[:, g : g + 1])

            # per-partition loss, then collapse partitions via ones-matmul
            lp1 = fin_pool.tile([P, 1], fp32, tag="lp1")
            nc.vector.tensor_reduce(out=lp1[:], in_=lp[:], axis=X, op=ADD)
            acc = ps_pool.tile([1, 1], fp32)
            nc.tensor.matmul(out=acc[:], lhsT=ones[:], rhs=lp1[:],
                             start=True, stop=True)
            out_sb = fin_pool.tile([1, 1], fp32, tag="out")
            nc.vector.tensor_copy(out=out_sb[:], in_=acc[:])
            nc.sync.dma_start(out=loss[:], in_=out_sb[:])

    nc.compile()
    return nc


def _get_nc():
    if "nc" not in _NC_CACHE:
        _NC_CACHE["nc"] = _build_bass()
    return _NC_CACHE["nc"]


def _make_in_maps(pos_u, pos_w, neg_w, u_emb, w_emb):
    pos_u = np.asarray(pos_u).astype(np.int32)
    pos_w = np.asarray(pos_w).astype(np.int32)
    neg_w = np.asarray(neg_w).astype(np.int32)
    u_emb = np.asarray(u_emb, dtype=np.float32)
    w_emb = np.asarray(w_emb, dtype=np.float32)

    fp8_np = mybir.dt.np(mybir.dt.float8e4)
    emb_cat = np.ascontiguousarray(
        np.clip(
            np.concatenate([u_emb * S_U, w_emb * S_W], axis=0), -224.0, 224.0
        ).astype(fp8_np)
    )

    starts = [sum(GROUPS[:g]) for g in range(len(GROUPS))]
    in_maps = []
    for i in range(N_CORES):
        sl = slice(i * B_LOC, (i + 1) * B_LOC)
        # batch row b -> (chunk c = b // 128, partition p = b % 128)
        rows_u = pos_u[sl].reshape(N_CHUNK, P, C)          # [c, p, k]
        rows_w = np.concatenate(
            [pos_w[sl, None] + V, neg_w[sl] + V], axis=1
        ).reshape(N_CHUNK, P, J)                           # [c, p, j]
        # per group: u-block k-major [p, k, c], then w-block [p, c, j]
        cols = []
        for g, n in enumerate(GROUPS):
            c0 = starts[g]
            cols.append(
                rows_u[c0 : c0 + n].transpose(1, 2, 0).reshape(P, C * n)
            )
            cols.append(
                rows_w[c0 : c0 + n].transpose(1, 0, 2).reshape(P, J * n)
            )
        gidx = np.concatenate(cols, axis=1)                # [P, N_CHUNK * R]
        in_maps.append(
            {
                "emb_cat": emb_cat,
                "gidx": np.ascontiguousarray(gidx),
            }
        )
    return in_maps


def _install_axon_profile_shim():
    """Provide antenv.axon_hooks (missing in this image) so trace=True can
    capture NTFF profiles via the axon PJRT .so, and keep trace artifacts
    local instead of uploading to a bucket."""
    import contextlib
    import ctypes
    import types

    import concourse.bass_utils as bu

    bu.upload_artifacts = lambda tmpdir: tmpdir

    try:
        from antenv.axon_hooks import get_axon_ntff_profile_hook  # noqa: F401

        return
    except ImportError:
        pass

    mod = types.ModuleType("antenv.axon_hooks")
    holder = {}
    mod.set_axon_ntff_profile_hook = lambda h: holder.__setitem__("h", h)
    mod.get_axon_ntff_profile_hook = lambda: holder.get("h")
    sys.modules["antenv.axon_hooks"] = mod
    import antenv

    antenv.axon_hooks = mod

    so_path = "/opt/axon/libaxon_pjrt.so"
    lib = ctypes.CDLL(so_path)
    if not hasattr(lib, "axon_start_nrt_profile"):
        return
    lib.axon_start_nrt_profile.argtypes = [
        ctypes.POINTER(ctypes.c_int64),
        ctypes.c_size_t,
    ]
    lib.axon_start_nrt_profile.restype = ctypes.c_int64
    lib.axon_stop_nrt_profile.argtypes = [ctypes.c_char_p]
    lib.axon_stop_nrt_profile.restype = ctypes.c_int64

    @contextlib.contextmanager
    def _hook(output_dir, device_ids):
        import jax

        jax.devices()
        if device_ids:
            ids = (ctypes.c_int64 * len(device_ids))(*device_ids)
            rc = lib.axon_start_nrt_profile(ids, len(device_ids))
        else:
            rc = lib.axon_start_nrt_profile(None, 0)
        if rc != 0:
            raise RuntimeError(f"axon_start_nrt_profile rc={rc}")
        try:
            yield
        finally:
            n = lib.axon_stop_nrt_profile(str(output_dir).encode())
            print(f"profile: {n} file(s) written to {output_dir}")

    mod.set_axon_ntff_profile_hook(_hook)


def _run(in_maps, trace=False):
    if trace:
        _install_axon_profile_shim()
    nc = _get_nc()
    return run_bass_kernel_spmd(nc, in_maps, list(range(N_CORES)), trace=trace)


def kernel(pos_u, pos_w, neg_w, u_emb, w_emb):
    in_maps = _make_in_maps(pos_u, pos_w, neg_w, u_emb, w_emb)
    bkr = _run(in_maps, trace=False)
    total = 0.0
    for r in bkr.results:
        total += float(r["loss_part"].astype(np.float64).sum())
    return np.float32(total)


def kernel_traced(pos_u, pos_w, neg_w, u_emb, w_emb):
    """Like kernel() but returns (loss, BassKernelResults) with HW profile."""
    in_maps = _make_in_maps(pos_u, pos_w, neg_w, u_emb, w_emb)
    bkr = _run(in_maps, trace=True)
    total = 0.0
    for r in bkr.results:
        total += float(r["loss_part"].astype(np.float64).sum())
    return np.float32(total), bkr


# revision 7
# speedup vs baseline: 1.3074x; 1.1829x over previous
"""CBOW negative-sampling loss on 8 Trainium2 NeuronCores.

Strategy: replicate the embedding tables, data-parallel over the batch dim.
Each core handles 2048 of the 16384 batch rows.

Host side: u_emb (x1024) and w_emb (x32) are concatenated into one [2V, D]
fp8-e4m3 table so each group needs a single indirect-DMA gather (w-indices
offset by +V). fp8 halves the HBM gather traffic vs bf16; the SWDGE casts
to bf16 on the fly during the gather so DVE keeps its 2x 16-bit rate. The
scale factors keep fp8 values in the normal range and are divided back out
in the activation's scale factor.

Per-core kernel layout:
  - batch row b -> chunk c = b // 128, partition p = b % 128.
  - 16 chunks in ramped groups; per group ONE indirect gather pulls, per
    partition, n_chunks x (8 u-rows + 6 w-rows) x 128 fp8 -> bf16. The
    group's u-rows are ordered k-major (all k=0 rows, then k=1, ...) so
    each k-slab is one contiguous [P, n*D] block.
  - h = sum of the 8 context embeddings: 8 accumulating identity-weight
    matmuls per group (rhs = the k-th slab) on the (otherwise idle)
    TensorEngine -> PSUM, then one ACT copy PSUM -> SBUF bf16. Frees
    ~7us of DVE adds.
  - dots on DVE: one broadcast-mult [P,n,6,128] (bf16, in-place over the
    w block), 3 contiguous fold-adds 128->16, TensorReduce -> f32 scores
    (pos negated so every loss term is softplus(+x)).
  - softplus per group on ACT: Exp then Ln(1+x) with accum_out -> lp[:, g].
    Both functions come from one activation table (id 6) preloaded at
    kernel start, so no mid-kernel table swaps.
  - finale: reduce lp rows, PE ones-matmul to collapse partitions -> [1,1],
    host sums the 8 per-core scalars.

loss = sum_b softplus(-score_b) + sum_{b,k} softplus(+neg_score_bk)
"""

import sys

import numpy as np

sys.path.insert(0, "/opt/trn_rl_repo")

from concourse import bacc, bass, mybir, tile  # noqa: E402
from concourse.bass_utils import run_bass_kernel_spmd  # noqa: E402
from concourse.masks import make_identity  # noqa: E402

V, D = 100000, 128
B, C, K = 16384, 8, 5
N_CORES = 8
P = 128
B_LOC = B // N_CORES            # 2048 batch rows per core
N_CHUNK = B_LOC // P            # 16 chunks of 128 rows
GROUPS = (1, 3, 4, 4, 3, 1)     # chunks per indirect-DMA gather group
assert sum(GROUPS) == N_CHUNK
# matmul moving operand and a PSUM bank both cap at 512 f32 per partition
assert max(GROUPS) * D <= 512
J = 1 + K                       # 6 w-rows per batch row (pos + negs)
R = C + J                       # 14 gathered rows per batch row

S_U = 1024.0                    # host-side scale into fp8 normal range
S_W = 32.0
ACT_SCALE = 1.0 / (S_U * S_W)   # divided back out inside the Exp

_NC_CACHE = {}


def _build_bass():
    nc = bacc.Bacc(
        "TRN2",
        target_bir_lowering=False,
        debug=False,
        dynamic_dma_scratch_size=65536,
    )

    bf16 = mybir.dt.bfloat16
    fp32 = mybir.dt.float32
    fp8 = mybir.dt.float8e4
    X = mybir.AxisListType.X
    ADD = mybir.AluOpType.add
    NG = len(GROUPS)
    EXP = mybir.ActivationFunctionType.Exp
    LN = mybir.ActivationFunctionType.Ln
    COPY = mybir.ActivationFunctionType.Copy

    emb = nc.dram_tensor("emb_cat", [2 * V, D], fp8, kind="ExternalInput")
    gidx = nc.dram_tensor(
        "gidx", [P, N_CHUNK * R], mybir.dt.int32, kind="ExternalInput"
    )
    loss = nc.dram_tensor("loss_part", [1, 1], fp32, kind="ExternalOutput")

    with tile.TileContext(nc) as tc:
        with (
            tc.tile_pool(name="idx", bufs=1) as idx_pool,
            tc.tile_pool(name="gb", bufs=5) as gb_pool,
            tc.tile_pool(name="sc", bufs=2) as sc_pool,
            tc.tile_pool(name="sp", bufs=2) as sp_pool,
            tc.tile_pool(name="fin", bufs=1) as fin_pool,
            tc.tile_pool(name="hp", bufs=2, space="PSUM") as hp_pool,
            tc.tile_pool(name="ps", bufs=1, space="PSUM") as ps_pool,
        ):
            # one table (id 6: natural_log_exp_and_others) serves Exp AND Ln
            nc.scalar.add_instruction(mybir.InstLoadActFuncSet(
                name=nc.get_next_instruction_name(), act_func_set_id=6,
                ins=[], outs=[]))

            ix_all = idx_pool.tile([P, N_CHUNK * R], mybir.dt.int32, tag="ix")
            n0 = GROUPS[0] * R
            nc.sync.dma_start(out=ix_all[:, 0:n0], in_=gidx[:, 0:n0])
            nc.sync.dma_start(out=ix_all[:, n0:], in_=gidx[:, n0:])

            ident = fin_pool.tile([P, P], bf16, tag="ident")
            make_identity(nc, ident[:])
            ones = fin_pool.tile([P, 1], fp32, tag="ones")
            nc.gpsimd.memset(ones[:], 1.0)

            h_sb = fin_pool.tile([P, N_CHUNK * D], bf16, tag="h_sb")
            h16 = h_sb[:].rearrange("p (c d) -> p c d", c=N_CHUNK)
            lp = fin_pool.tile([P, NG], fp32, tag="lp")

            starts = [sum(GROUPS[:g]) for g in range(NG)]
            gb_t = {}

            def issue_gather(g):
                n = GROUPS[g]
                c0 = starts[g]
                gb = gb_pool.tile([P, n * R * D], bf16, tag="gb")
                nc.gpsimd.indirect_dma_start(
                    out=gb[:],
                    out_offset=None,
                    in_=emb[:],
                    in_offset=bass.IndirectOffsetOnAxis(
                        ap=ix_all[:, c0 * R : (c0 + n) * R], axis=0
                    ),
                )
                gb_t[g] = gb

            issue_gather(0)
            for g in range(NG):
                if g + 1 < NG:
                    issue_gather(g + 1)
                n = GROUPS[g]
                c0 = starts[g]
                gb = gb_t.pop(g)

                # h = sum of the 8 context embeddings, on the TensorEngine:
                # 8 accumulating identity matmuls (rhs = the k-th u-slab,
                # contiguous [P, n*D]) -> PSUM [P, n*D] f32.
                hp = hp_pool.tile([P, n * D], fp32, tag="hp")
                for k in range(C):
                    nc.tensor.matmul(
                        out=hp[:],
                        lhsT=ident[:],
                        rhs=gb[:, k * n * D : (k + 1) * n * D],
                        start=(k == 0),
                        stop=(k == C - 1),
                    )
                # PSUM f32 -> SBUF bf16 on the Scalar engine (idle otherwise)
                nc.scalar.activation(
                    out=h_sb[:, c0 * D : (c0 + n) * D], in_=hp[:], func=COPY
                )

                # m[p, c, j, d] = w[p, c, j, d] * h[p, c, d], in place over w
                w4 = gb[:, C * n * D : R * n * D].rearrange(
                    "p (c j d) -> p c j d", c=n, j=J
                )
                h4 = h16[:, c0 : c0 + n]
                nc.vector.scalar_tensor_tensor(
                    out=w4[:, :, 0, :],
                    in0=w4[:, :, 0, :],
                    scalar=-1.0,
                    in1=h4,
                    op0=mybir.AluOpType.mult,
                    op1=mybir.AluOpType.mult,
                )
                nc.vector.tensor_mul(
                    out=w4[:, :, 1:J, :],
                    in0=w4[:, :, 1:J, :],
                    in1=h4[:, :, None, :].broadcast_to([P, n, J - 1, D]),
                )
                # fold the innermost 128 -> 16 with bf16 adds before the
                # (1x-rate) TensorReduce
                for w_ in (64, 32, 16):
                    nc.vector.tensor_add(
                        out=w4[:, :, :, 0:w_],
                        in0=w4[:, :, :, 0:w_],
                        in1=w4[:, :, :, w_ : 2 * w_],
                    )
                # raw dots (f32), sign-flipped so every term is softplus(+x)
                sc = sc_pool.tile([P, n * J], fp32, tag="sc")
                sc3 = sc[:].rearrange("p (c j) -> p c j", j=J)
                nc.vector.tensor_reduce(
                    out=sc3, in_=w4[:, :, :, 0:16], axis=X, op=ADD,
                )
                # softplus(x) = ln(1 + exp(x)); accumulate into lp[:, g]
                sp = sp_pool.tile([P, n * J], fp32, tag="sp")
                nc.scalar.activation(out=sp[:], in_=sc[:], func=EXP,
                                     scale=ACT_SCALE)
                nc.scalar.activation(out=sp[:], in_=sp[:], func=LN, bias=1.0,
                                     accum_out=lp[:, g : g + 1])

            # per-partition loss, then collapse partitions via ones-matmul
            lp1 = fin_pool.tile([P, 1], fp32, tag="lp1")
            nc.vector.tensor_reduce(out=lp1[:], in_=lp[:], axis=X, op=ADD)
            acc = ps_pool.tile([1, 1], fp32)
            nc.tensor.matmul(out=acc[:], lhsT=ones[:], rhs=lp1[:],
                             start=True, stop=True)
            out_sb = fin_pool.tile([1, 1], fp32, tag="out")
            nc.vector.tensor_copy(out=out_sb[:], in_=acc[:])
            nc.sync.dma_start(out=loss[:], in_=out_sb[:])

    nc.compile()
    return nc


def _get_nc():
    if "nc" not in _NC_CACHE:
        _NC_CACHE["nc"] = _build_bass()
    return _NC_CACHE["nc"]


def _make_in_maps(pos_u, pos_w, neg_w, u_emb, w_emb):
    pos_u = np.asarray(pos_u).astype(np.int32)
    pos_w = np.asarray(pos_w).astype(np.int32)
    neg_w = np.asarray(neg_w).astype(np.int32)
    u_emb = np.asarray(u_emb, dtype=np.float32)
    w_emb = np.asarray(w_emb, dtype=np.float32)

    fp8_np = mybir.dt.np(mybir.dt.float8e4)
    emb_cat = np.ascontiguousarray(
        np.clip(
            np.concatenate([u_emb * S_U, w_emb * S_W], axis=0), -224.0, 224.0
        ).astype(fp8_np)
    )

    starts = [sum(GROUPS[:g]) for g in range(len(GROUPS))]
    in_maps = []
    for i in range(N_CORES):
        sl = slice(i * B_LOC, (i + 1) * B_LOC)
        # batch row b -> (chunk c = b // 128, partition p = b % 128)
        rows_u = pos_u[sl].reshape(N_CHUNK, P, C)          # [c, p, k]
        rows_w = np.concatenate(
            [pos_w[sl, None] + V, neg_w[sl] + V], axis=1
        ).reshape(N_CHUNK, P, J)                           # [c, p, j]
        # per group: u-block k-major [p, k, c], then w-block [p, c, j]
        cols = []
        for g, n in enumerate(GROUPS):
            c0 = starts[g]
            cols.append(
                rows_u[c0 : c0 + n].transpose(1, 2, 0).reshape(P, C * n)
            )
            cols.append(
                rows_w[c0 : c0 + n].transpose(1, 0, 2).reshape(P, J * n)
            )
        gidx = np.concatenate(cols, axis=1)                # [P, N_CHUNK * R]
        in_maps.append(
            {
                "emb_cat": emb_cat,
                "gidx": np.ascontiguousarray(gidx),
            }
        )
    return in_maps


def _install_axon_profile_shim():
    """Provide antenv.axon_hooks (missing in this image) so trace=True can
    capture NTFF profiles via the axon PJRT .so, and keep trace artifacts
    local instead of uploading to a bucket."""
    import contextlib
    import ctypes
    import types

    import concourse.bass_utils as bu

    bu.upload_artifacts = lambda tmpdir: tmpdir

    try:
        from antenv.axon_hooks import get_axon_ntff_profile_hook  # noqa: F401

        return
    except ImportError:
        pass

    mod = types.ModuleType("antenv.axon_hooks")
    holder = {}
    mod.set_axon_ntff_profile_hook = lambda h: holder.__setitem__("h", h)
    mod.get_axon_ntff_profile_hook = lambda: holder.get("h")
    sys.modules["antenv.axon_hooks"] = mod
    import antenv

    antenv.axon_hooks = mod

    so_path = "/opt/axon/libaxon_pjrt.so"
    lib = ctypes.CDLL(so_path)
    if not hasattr(lib, "axon_start_nrt_profile"):
        return
    lib.axon_start_nrt_profile.argtypes = [
        ctypes.POINTER(ctypes.c_int64),
        ctypes.c_size_t,
    ]
    lib.axon_start_nrt_profile.restype = ctypes.c_int64
    lib.axon_stop_nrt_profile.argtypes = [ctypes.c_char_p]
    lib.axon_stop_nrt_profile.restype = ctypes.c_int64

    @contextlib.contextmanager
    def _hook(output_dir, device_ids):
        import jax

        jax.devices()
        if device_ids:
            ids = (ctypes.c_int64 * len(device_ids))(*device_ids)
            rc = lib.axon_start_nrt_profile(ids, len(device_ids))
        else:
            rc = lib.axon_start_nrt_profile(None, 0)
        if rc != 0:
            raise RuntimeError(f"axon_start_nrt_profile rc={rc}")
        try:
            yield
        finally:
            n = lib.axon_stop_nrt_profile(str(output_dir).encode())
            print(f"profile: {n} file(s) written to {output_dir}")

    mod.set_axon_ntff_profile_hook(_hook)


def _run(in_maps, trace=False):
    if trace:
        _install_axon_profile_shim()
    nc = _get_nc()
    return run_bass_kernel_spmd(nc, in_maps, list(range(N_CORES)), trace=trace)


def kernel(pos_u, pos_w, neg_w, u_emb, w_emb):
    in_maps = _make_in_maps(pos_u, pos_w, neg_w, u_emb, w_emb)
    bkr = _run(in_maps, trace=False)
    total = 0.0
    for r in bkr.results:
        total += float(r["loss_part"].astype(np.float64).sum())
    return np.float32(total)


def kernel_traced(pos_u, pos_w, neg_w, u_emb, w_emb):
    """Like kernel() but returns (loss, BassKernelResults) with HW profile."""
    in_maps = _make_in_maps(pos_u, pos_w, neg_w, u_emb, w_emb)
    bkr = _run(in_maps, trace=True)
    total = 0.0
    for r in bkr.results:
        total += float(r["loss_part"].astype(np.float64).sum())
    return np.float32(total), bkr


# revision 8
# speedup vs baseline: 1.3649x; 1.0440x over previous
"""CBOW negative-sampling loss on 8 Trainium2 NeuronCores.

Strategy: replicate the embedding tables, data-parallel over the batch dim.
Each core handles 2048 of the 16384 batch rows.

Host side: u_emb (x1024) and w_emb (x32) are concatenated into one [2V, D]
fp8-e4m3 table (w-indices offset by +V). The gather traffic is descriptor-
rate-bound (~10ns/row/engine), so the layout splits u and w gathers:
  - u-rows are gathered raw fp8 (no cast) and consumed by the TensorEngine,
    which handles fp8 natively;
  - w-rows are gathered with an on-the-fly SWDGE cast to bf16 so the DVE
    keeps its 2x 16-bit rate for the score products.
The scale factors keep fp8 values in the normal range and are divided back
out in the activation's scale factor.

Per-core kernel layout:
  - batch row b -> chunk c = b // 128, partition p = b % 128.
  - 16 chunks in ramped groups. Per group one u-gather (k-major slabs: all
    k=0 rows, then k=1, ...) and one w-gather; w0+w1 share one gather and
    the last group uses a single combined cast gather. Transfer order is
    tuned so the first w block lands right after u0 (DVE starts early).
  - h = sum of the 8 context embeddings: 8 accumulating identity-weight
    matmuls per group (rhs = the k-th fp8 slab) on the TensorEngine ->
    PSUM f32, then one ACT copy PSUM -> SBUF bf16. The last (1-chunk)
    group sums on DVE instead to shorten the tail chain.
  - dots on DVE: j=0 products via scalar_tensor_tensor with scalar=-1 (the
    sign fold makes every loss term softplus(+x)), j=1..5 via one
    broadcast-mult, 3 contiguous fold-adds 128->16, one TensorReduce.
  - softplus per group on ACT: Exp then Ln(1+x) with accum_out -> lp[:, g].
    Both functions come from one activation table (id 6) preloaded at
    kernel start, so no mid-kernel table swaps.
  - finale: reduce lp rows, PE ones-matmul to collapse partitions -> [1,1],
    host sums the 8 per-core scalars.

loss = sum_b softplus(-score_b) + sum_{b,k} softplus(+neg_score_bk)
"""

import sys

import numpy as np

sys.path.insert(0, "/opt/trn_rl_repo")

from concourse import bacc, bass, mybir, tile  # noqa: E402
from concourse.bass_utils import run_bass_kernel_spmd  # noqa: E402
from concourse.masks import make_identity  # noqa: E402

V, D = 100000, 128
B, C, K = 16384, 8, 5
N_CORES = 8
P = 128
B_LOC = B // N_CORES            # 2048 batch rows per core
N_CHUNK = B_LOC // P            # 16 chunks of 128 rows
GROUPS = (1, 3, 4, 4, 3, 1)     # chunks per gather group; last is combined
assert sum(GROUPS) == N_CHUNK
# matmul moving operand and a PSUM bank both cap at 512 f32 per partition
assert max(GROUPS) * D <= 512
J = 1 + K                       # 6 w-rows per batch row (pos + negs)
R = C + J                       # 14 gathered rows per batch row
NG = len(GROUPS)
NPE = NG - 1                    # groups whose h-sum runs on the PE
STARTS = [sum(GROUPS[:g]) for g in range(NG)]
# gidx column layout: [u0k .. u4k | w0 .. w4 | u5k w5]
U_OFF = [sum(C * n for n in GROUPS[:g]) for g in range(NPE)]
W_BASE = sum(C * n for n in GROUPS[:NPE])
W_OFF = [W_BASE + sum(J * n for n in GROUPS[:g]) for g in range(NPE)]
G5_OFF = W_BASE + sum(J * n for n in GROUPS[:NPE])
assert G5_OFF + R * GROUPS[NPE] == N_CHUNK * R

S_U = 1024.0                    # host-side scale into fp8 normal range
S_W = 32.0
ACT_SCALE = 1.0 / (S_U * S_W)   # divided back out inside the Exp

_NC_CACHE = {}


def _build_bass():
    nc = bacc.Bacc(
        "TRN2",
        target_bir_lowering=False,
        debug=False,
        dynamic_dma_scratch_size=65536,
    )

    bf16 = mybir.dt.bfloat16
    fp32 = mybir.dt.float32
    fp8 = mybir.dt.float8e4
    X = mybir.AxisListType.X
    ADD = mybir.AluOpType.add
    MUL = mybir.AluOpType.mult
    EXP = mybir.ActivationFunctionType.Exp
    LN = mybir.ActivationFunctionType.Ln
    COPY = mybir.ActivationFunctionType.Copy

    emb = nc.dram_tensor("emb_cat", [2 * V, D], fp8, kind="ExternalInput")
    gidx = nc.dram_tensor(
        "gidx", [P, N_CHUNK * R], mybir.dt.int32, kind="ExternalInput"
    )
    loss = nc.dram_tensor("loss_part", [1, 1], fp32, kind="ExternalOutput")

    with tile.TileContext(nc) as tc:
        with (
            tc.tile_pool(name="idx", bufs=1) as idx_pool,
            tc.tile_pool(name="ub", bufs=5) as ub_pool,
            tc.tile_pool(name="wb", bufs=5) as wb_pool,
            tc.tile_pool(name="sc", bufs=2) as sc_pool,
            tc.tile_pool(name="sp", bufs=2) as sp_pool,
            tc.tile_pool(name="fin", bufs=1) as fin_pool,
            tc.tile_pool(name="hp", bufs=3, space="PSUM") as hp_pool,
            tc.tile_pool(name="ps", bufs=1, space="PSUM") as ps_pool,
        ):
            # one table (id 6: natural_log_exp_and_others) serves Exp AND Ln
            nc.scalar.add_instruction(mybir.InstLoadActFuncSet(
                name=nc.get_next_instruction_name(), act_func_set_id=6,
                ins=[], outs=[]))

            ix_all = idx_pool.tile([P, N_CHUNK * R], mybir.dt.int32, tag="ix")
            n0 = C * GROUPS[0]   # u0 columns only: first desc-gen starts ASAP
            nc.sync.dma_start(out=ix_all[:, 0:n0], in_=gidx[:, 0:n0])
            nc.sync.dma_start(out=ix_all[:, n0:], in_=gidx[:, n0:])

            ident = fin_pool.tile([P, P], fp8, tag="ident")
            make_identity(nc, ident[:])
            ones = fin_pool.tile([P, 1], fp32, tag="ones")
            nc.gpsimd.memset(ones[:], 1.0)

            h_sb = fin_pool.tile([P, N_CHUNK * D], bf16, tag="h_sb")
            h16 = h_sb[:].rearrange("p (c d) -> p c d", c=N_CHUNK)
            lp = fin_pool.tile([P, NG], fp32, tag="lp")

            def gather(cols, ncols, dtype, tag, pool):
                t = pool.tile([P, ncols * D], dtype, tag=tag)
                nc.gpsimd.indirect_dma_start(
                    out=t[:],
                    out_offset=None,
                    in_=emb[:],
                    in_offset=bass.IndirectOffsetOnAxis(
                        ap=ix_all[:, cols : cols + ncols], axis=0
                    ),
                )
                return t

            # issue all gathers up front; the list order is the transfer
            # order: u0 | w0+w1 | u1 u2 | w2 | u3 | w3 | u4 | w4 | u5+w5
            ub, wbt = {}, {}
            ub[0] = gather(U_OFF[0], C * GROUPS[0], fp8, "ub", ub_pool)
            w01 = gather(W_OFF[0], J * (GROUPS[0] + GROUPS[1]), bf16, "wb",
                         wb_pool)
            wbt[0] = w01[:, 0 : J * GROUPS[0] * D]
            wbt[1] = w01[:, J * GROUPS[0] * D :]
            ub[1] = gather(U_OFF[1], C * GROUPS[1], fp8, "ub", ub_pool)
            ub[2] = gather(U_OFF[2], C * GROUPS[2], fp8, "ub", ub_pool)
            wbt[2] = gather(W_OFF[2], J * GROUPS[2], bf16, "wb", wb_pool)[:]
            ub[3] = gather(U_OFF[3], C * GROUPS[3], fp8, "ub", ub_pool)
            wbt[3] = gather(W_OFF[3], J * GROUPS[3], bf16, "wb", wb_pool)[:]
            ub[4] = gather(U_OFF[4], C * GROUPS[4], fp8, "ub", ub_pool)
            wbt[4] = gather(W_OFF[4], J * GROUPS[4], bf16, "wb", wb_pool)[:]
            g5 = gather(G5_OFF, R * GROUPS[NPE], bf16, "g5", wb_pool)
            ub[NPE] = g5[:, 0 : C * GROUPS[NPE] * D]
            wbt[NPE] = g5[:, C * GROUPS[NPE] * D :]

            for g in range(NG):
                n = GROUPS[g]
                c0 = STARTS[g]
                if g < NPE:
                    # h-sum on the TensorEngine: 8 accumulating identity
                    # matmuls (rhs = the k-th fp8 u-slab) -> PSUM f32
                    hp = hp_pool.tile([P, n * D], fp32, tag="hp")
                    for k in range(C):
                        nc.tensor.matmul(
                            out=hp[:],
                            lhsT=ident[:],
                            rhs=ub[g][:, k * n * D : (k + 1) * n * D],
                            start=(k == 0),
                            stop=(k == C - 1),
                        )
                    # PSUM f32 -> SBUF bf16 on the Scalar engine
                    nc.scalar.activation(
                        out=h_sb[:, c0 * D : (c0 + n) * D], in_=hp[:],
                        func=COPY,
                    )
                else:
                    # tail group: binary-tree h-sum on DVE over the k-major
                    # bf16 slabs, last fold writes straight into h_sb
                    u5 = ub[g]
                    nc.vector.tensor_add(
                        out=u5[:, 0 : 4 * n * D], in0=u5[:, 0 : 4 * n * D],
                        in1=u5[:, 4 * n * D : 8 * n * D],
                    )
                    nc.vector.tensor_add(
                        out=u5[:, 0 : 2 * n * D], in0=u5[:, 0 : 2 * n * D],
                        in1=u5[:, 2 * n * D : 4 * n * D],
                    )
                    nc.vector.tensor_add(
                        out=h_sb[:, c0 * D : (c0 + n) * D],
                        in0=u5[:, 0 : n * D], in1=u5[:, n * D : 2 * n * D],
                    )

                # m[p, c, j, d] = w[p, c, j, d] * h[p, c, d], in place over w
                w4 = wbt[g].rearrange("p (c j d) -> p c j d", c=n, j=J)
                h4 = h16[:, c0 : c0 + n]
                nc.vector.scalar_tensor_tensor(
                    out=w4[:, :, 0, :], in0=w4[:, :, 0, :], scalar=-1.0,
                    in1=h4, op0=MUL, op1=MUL,
                )
                nc.vector.tensor_mul(
                    out=w4[:, :, 1:J, :],
                    in0=w4[:, :, 1:J, :],
                    in1=h4[:, :, None, :].broadcast_to([P, n, J - 1, D]),
                )
                # fold the innermost 128 -> 16 with bf16 adds before the
                # (1x-rate) TensorReduce
                for w_ in (64, 32, 16):
                    nc.vector.tensor_add(
                        out=w4[:, :, :, 0:w_],
                        in0=w4[:, :, :, 0:w_],
                        in1=w4[:, :, :, w_ : 2 * w_],
                    )
                # raw dots (f32); j=0 was sign-folded in the multiply
                sc = sc_pool.tile([P, n * J], fp32, tag="sc")
                sc3 = sc[:].rearrange("p (c j) -> p c j", j=J)
                nc.vector.tensor_reduce(
                    out=sc3, in_=w4[:, :, :, 0:16], axis=X, op=ADD,
                )
                # softplus(x) = ln(1 + exp(x)); accumulate into lp[:, g]
                sp = sp_pool.tile([P, n * J], fp32, tag="sp")
                nc.scalar.activation(out=sp[:], in_=sc[:], func=EXP,
                                     scale=ACT_SCALE)
                nc.scalar.activation(out=sp[:], in_=sp[:], func=LN, bias=1.0,
                                     accum_out=lp[:, g : g + 1])

            # per-partition loss, then collapse partitions via ones-matmul
            lp1 = fin_pool.tile([P, 1], fp32, tag="lp1")
            nc.vector.tensor_reduce(out=lp1[:], in_=lp[:], axis=X, op=ADD)
            acc = ps_pool.tile([1, 1], fp32)
            nc.tensor.matmul(out=acc[:], lhsT=ones[:], rhs=lp1[:],
                             start=True, stop=True)
            out_sb = fin_pool.tile([1, 1], fp32, tag="out")
            nc.vector.tensor_copy(out=out_sb[:], in_=acc[:])
            nc.sync.dma_start(out=loss[:], in_=out_sb[:])

    nc.compile()
    return nc


def _get_nc():
    if "nc" not in _NC_CACHE:
        _NC_CACHE["nc"] = _build_bass()
    return _NC_CACHE["nc"]


def _make_in_maps(pos_u, pos_w, neg_w, u_emb, w_emb):
    pos_u = np.asarray(pos_u).astype(np.int32)
    pos_w = np.asarray(pos_w).astype(np.int32)
    neg_w = np.asarray(neg_w).astype(np.int32)
    u_emb = np.asarray(u_emb, dtype=np.float32)
    w_emb = np.asarray(w_emb, dtype=np.float32)

    fp8_np = mybir.dt.np(mybir.dt.float8e4)
    emb_cat = np.ascontiguousarray(
        np.clip(
            np.concatenate([u_emb * S_U, w_emb * S_W], axis=0), -224.0, 224.0
        ).astype(fp8_np)
    )

    in_maps = []
    for i in range(N_CORES):
        sl = slice(i * B_LOC, (i + 1) * B_LOC)
        # batch row b -> (chunk c = b // 128, partition p = b % 128)
        rows_u = pos_u[sl].reshape(N_CHUNK, P, C)          # [c, p, k]
        rows_w = np.concatenate(
            [pos_w[sl, None] + V, neg_w[sl] + V], axis=1
        ).reshape(N_CHUNK, P, J)                           # [c, p, j]
        # columns: [u0k .. u4k | w0 .. w4 | u5k w5]; u blocks k-major
        cols = []
        for g in range(NPE):
            c0, n = STARTS[g], GROUPS[g]
            cols.append(rows_u[c0 : c0 + n].transpose(1, 2, 0).reshape(P, -1))
        for g in range(NPE):
            c0, n = STARTS[g], GROUPS[g]
            cols.append(rows_w[c0 : c0 + n].transpose(1, 0, 2).reshape(P, -1))
        c0, n = STARTS[NPE], GROUPS[NPE]
        cols.append(rows_u[c0 : c0 + n].transpose(1, 2, 0).reshape(P, -1))
        cols.append(rows_w[c0 : c0 + n].transpose(1, 0, 2).reshape(P, -1))
        gidx = np.concatenate(cols, axis=1)                # [P, N_CHUNK * R]
        in_maps.append(
            {
                "emb_cat": emb_cat,
                "gidx": np.ascontiguousarray(gidx),
            }
        )
    return in_maps


def _install_axon_profile_shim():
    """Provide antenv.axon_hooks (missing in this image) so trace=True can
    capture NTFF profiles via the axon PJRT .so, and keep trace artifacts
    local instead of uploading to a bucket."""
    import contextlib
    import ctypes
    import types

    import concourse.bass_utils as bu

    bu.upload_artifacts = lambda tmpdir: tmpdir

    try:
        from antenv.axon_hooks import get_axon_ntff_profile_hook  # noqa: F401

        return
    except ImportError:
        pass

    mod = types.ModuleType("antenv.axon_hooks")
    holder = {}
    mod.set_axon_ntff_profile_hook = lambda h: holder.__setitem__("h", h)
    mod.get_axon_ntff_profile_hook = lambda: holder.get("h")
    sys.modules["antenv.axon_hooks"] = mod
    import antenv

    antenv.axon_hooks = mod

    so_path = "/opt/axon/libaxon_pjrt.so"
    lib = ctypes.CDLL(so_path)
    if not hasattr(lib, "axon_start_nrt_profile"):
        return
    lib.axon_start_nrt_profile.argtypes = [
        ctypes.POINTER(ctypes.c_int64),
        ctypes.c_size_t,
    ]
    lib.axon_start_nrt_profile.restype = ctypes.c_int64
    lib.axon_stop_nrt_profile.argtypes = [ctypes.c_char_p]
    lib.axon_stop_nrt_profile.restype = ctypes.c_int64

    @contextlib.contextmanager
    def _hook(output_dir, device_ids):
        import jax

        jax.devices()
        if device_ids:
            ids = (ctypes.c_int64 * len(device_ids))(*device_ids)
            rc = lib.axon_start_nrt_profile(ids, len(device_ids))
        else:
            rc = lib.axon_start_nrt_profile(None, 0)
        if rc != 0:
            raise RuntimeError(f"axon_start_nrt_profile rc={rc}")
        try:
            yield
        finally:
            n = lib.axon_stop_nrt_profile(str(output_dir).encode())
            print(f"profile: {n} file(s) written to {output_dir}")

    mod.set_axon_ntff_profile_hook(_hook)


def _run(in_maps, trace=False):
    if trace:
        _install_axon_profile_shim()
    nc = _get_nc()
    return run_bass_kernel_spmd(nc, in_maps, list(range(N_CORES)), trace=trace)


def kernel(pos_u, pos_w, neg_w, u_emb, w_emb):
    in_maps = _make_in_maps(pos_u, pos_w, neg_w, u_emb, w_emb)
    bkr = _run(in_maps, trace=False)
    total = 0.0
    for r in bkr.results:
        total += float(r["loss_part"].astype(np.float64).sum())
    return np.float32(total)


def kernel_traced(pos_u, pos_w, neg_w, u_emb, w_emb):
    """Like kernel() but returns (loss, BassKernelResults) with HW profile."""
    in_maps = _make_in_maps(pos_u, pos_w, neg_w, u_emb, w_emb)
    bkr = _run(in_maps, trace=True)
    total = 0.0
    for r in bkr.results:
        total += float(r["loss_part"].astype(np.float64).sum())
    return np.float32(total), bkr


# revision 10
# speedup vs baseline: 1.4088x; 1.0321x over previous
"""CBOW negative-sampling loss on 8 Trainium2 NeuronCores.

Strategy: replicate the embedding tables, data-parallel over the batch dim.
Each core handles 2048 of the 16384 batch rows.

Host side: u_emb (x1024), w_emb (x32) and -w_emb are concatenated into one
[3V, D] fp8-e4m3 table (neg_w indices offset by +V, pos_w by +2V into the
negated copy, so every score product already carries its loss sign and the
dot reduce needs no sign handling). The gather traffic is descriptor-rate-
bound (~10ns/row/engine), so the layout splits u and w gathers:
  - u-rows are gathered raw fp8 (no cast) and consumed by the TensorEngine,
    which handles fp8 natively;
  - w-rows are gathered with an on-the-fly SWDGE cast to bf16 so the DVE
    keeps its 2x 16-bit rate for the score products.
The scale factors keep fp8 values in the normal range and are divided back
out in the activation's scale factor.

Per-core kernel layout:
  - batch row b -> chunk c = b // 128, partition p = b % 128.
  - 16 chunks in ramped groups. Per group one u-gather (k-major slabs: all
    k=0 rows, then k=1, ...) and one w-gather; w0+w1 share one gather and
    the last group uses a single combined cast gather. Transfer order is
    tuned so the first w block lands right after u0 (DVE starts early).
  - h = sum of the 8 context embeddings: 8 accumulating identity-weight
    matmuls per group (rhs = the k-th fp8 slab) on the TensorEngine ->
    PSUM f32, then one ACT copy PSUM -> SBUF bf16. The last (1-chunk)
    group sums on DVE instead to shorten the tail chain.
  - dots on DVE: one broadcast-mult (2x bf16 rate), 3 contiguous fold-adds
    128->16, one TensorReduce -> softplus(+x) terms for every column.
  - the TensorEngine's HAM activity window is kept hot with filler matmuls
    so the h-sum runs at 2.4 GHz instead of the cold 1.2 GHz.
  - softplus per group on ACT: Exp then Ln(1+x) with accum_out -> lp[:, g].
    Both functions come from one activation table (id 6) preloaded at
    kernel start, so no mid-kernel table swaps.
  - finale: reduce lp rows, PE ones-matmul to collapse partitions -> [1,1],
    host sums the 8 per-core scalars.

loss = sum_b softplus(-score_b) + sum_{b,k} softplus(+neg_score_bk)
"""

import sys

import numpy as np

sys.path.insert(0, "/opt/trn_rl_repo")

from concourse import bacc, bass, mybir, tile  # noqa: E402
from concourse.bass_utils import run_bass_kernel_spmd  # noqa: E402
from concourse.masks import make_identity  # noqa: E402

V, D = 100000, 128
B, C, K = 16384, 8, 5
N_CORES = 8
P = 128
B_LOC = B // N_CORES            # 2048 batch rows per core
N_CHUNK = B_LOC // P            # 16 chunks of 128 rows
GROUPS = (1, 3, 4, 4, 3, 1)     # chunks per gather group; last is combined
assert sum(GROUPS) == N_CHUNK
# matmul moving operand and a PSUM bank both cap at 512 f32 per partition
assert max(GROUPS) * D <= 512
J = 1 + K                       # 6 w-rows per batch row (pos + negs)
R = C + J                       # 14 gathered rows per batch row
NG = len(GROUPS)
NPE = NG - 1                    # groups whose h-sum runs on the PE
STARTS = [sum(GROUPS[:g]) for g in range(NG)]
# gidx column layout: [u0k .. u4k | w0 .. w4 | u5k w5]
U_OFF = [sum(C * n for n in GROUPS[:g]) for g in range(NPE)]
W_BASE = sum(C * n for n in GROUPS[:NPE])
W_OFF = [W_BASE + sum(J * n for n in GROUPS[:g]) for g in range(NPE)]
G5_OFF = W_BASE + sum(J * n for n in GROUPS[:NPE])
assert G5_OFF + R * GROUPS[NPE] == N_CHUNK * R

S_U = 1024.0                    # host-side scale into fp8 normal range
S_W = 32.0
ACT_SCALE = 1.0 / (S_U * S_W)   # divided back out inside the Exp

_NC_CACHE = {}


def _build_bass():
    nc = bacc.Bacc(
        "TRN2",
        target_bir_lowering=False,
        debug=False,
        dynamic_dma_scratch_size=65536,
    )

    bf16 = mybir.dt.bfloat16
    fp32 = mybir.dt.float32
    fp8 = mybir.dt.float8e4
    X = mybir.AxisListType.X
    ADD = mybir.AluOpType.add
    MUL = mybir.AluOpType.mult
    EXP = mybir.ActivationFunctionType.Exp
    LN = mybir.ActivationFunctionType.Ln
    COPY = mybir.ActivationFunctionType.Copy

    emb = nc.dram_tensor("emb_cat", [3 * V, D], fp8, kind="ExternalInput")
    gidx = nc.dram_tensor(
        "gidx", [P, N_CHUNK * R], mybir.dt.int32, kind="ExternalInput"
    )
    loss = nc.dram_tensor("loss_part", [1, 1], fp32, kind="ExternalOutput")

    with tile.TileContext(nc) as tc:
        with (
            tc.tile_pool(name="idx", bufs=1) as idx_pool,
            tc.tile_pool(name="ub", bufs=5) as ub_pool,
            tc.tile_pool(name="wb", bufs=5) as wb_pool,
            tc.tile_pool(name="sc", bufs=2) as sc_pool,
            tc.tile_pool(name="sp", bufs=2) as sp_pool,
            tc.tile_pool(name="fin", bufs=1) as fin_pool,
            tc.tile_pool(name="hp", bufs=3, space="PSUM") as hp_pool,
            tc.tile_pool(name="ps", bufs=1, space="PSUM") as ps_pool,
        ):
            # one table (id 6: natural_log_exp_and_others) serves Exp AND Ln
            nc.scalar.add_instruction(mybir.InstLoadActFuncSet(
                name=nc.get_next_instruction_name(), act_func_set_id=6,
                ins=[], outs=[]))

            ix_all = idx_pool.tile([P, N_CHUNK * R], mybir.dt.int32, tag="ix")
            n0 = C * GROUPS[0]   # u0 columns only: first desc-gen starts ASAP
            nc.sync.dma_start(out=ix_all[:, 0:n0], in_=gidx[:, 0:n0])
            nc.sync.dma_start(out=ix_all[:, n0:], in_=gidx[:, n0:])

            ident = fin_pool.tile([P, P], fp8, tag="ident")
            make_identity(nc, ident[:])
            ones = fin_pool.tile([P, 1], fp32, tag="ones")
            nc.gpsimd.memset(ones[:], 1.0)

            warm = ps_pool.tile([P, P], fp32, tag="warm")

            def pe_filler(count):
                # keep the TensorEngine's activity window hot; results unused
                for _ in range(count):
                    nc.tensor.matmul(out=warm[:], lhsT=ident[:],
                                     rhs=ident[:], start=True, stop=True)

            pe_filler(45)

            h_sb = fin_pool.tile([P, N_CHUNK * D], bf16, tag="h_sb")
            h16 = h_sb[:].rearrange("p (c d) -> p c d", c=N_CHUNK)
            lp = fin_pool.tile([P, NG], fp32, tag="lp")

            def gather(cols, ncols, dtype, tag, pool):
                t = pool.tile([P, ncols * D], dtype, tag=tag)
                nc.gpsimd.indirect_dma_start(
                    out=t[:],
                    out_offset=None,
                    in_=emb[:],
                    in_offset=bass.IndirectOffsetOnAxis(
                        ap=ix_all[:, cols : cols + ncols], axis=0
                    ),
                )
                return t

            # issue all gathers up front; the list order is the transfer
            # order: u0 | w0+w1 | u1 u2 | w2 | u3 | w3 | u4 | w4 | u5+w5
            ub, wbt = {}, {}
            ub[0] = gather(U_OFF[0], C * GROUPS[0], fp8, "ub", ub_pool)
            w01 = gather(W_OFF[0], J * (GROUPS[0] + GROUPS[1]), bf16, "wb",
                         wb_pool)
            wbt[0] = w01[:, 0 : J * GROUPS[0] * D]
            wbt[1] = w01[:, J * GROUPS[0] * D :]
            ub[1] = gather(U_OFF[1], C * GROUPS[1], fp8, "ub", ub_pool)
            ub[2] = gather(U_OFF[2], C * GROUPS[2], fp8, "ub", ub_pool)
            wbt[2] = gather(W_OFF[2], J * GROUPS[2], bf16, "wb", wb_pool)[:]
            ub[3] = gather(U_OFF[3], C * GROUPS[3], fp8, "ub", ub_pool)
            wbt[3] = gather(W_OFF[3], J * GROUPS[3], bf16, "wb", wb_pool)[:]
            ub[4] = gather(U_OFF[4], C * GROUPS[4], fp8, "ub", ub_pool)
            wbt[4] = gather(W_OFF[4], J * GROUPS[4], bf16, "wb", wb_pool)[:]
            g5 = gather(G5_OFF, R * GROUPS[NPE], bf16, "g5", wb_pool)
            ub[NPE] = g5[:, 0 : C * GROUPS[NPE] * D]
            wbt[NPE] = g5[:, C * GROUPS[NPE] * D :]

            for g in range(NG):
                n = GROUPS[g]
                c0 = STARTS[g]
                if g < NPE:
                    # h-sum on the TensorEngine: 8 accumulating identity
                    # matmuls (rhs = the k-th fp8 u-slab) -> PSUM f32
                    hp = hp_pool.tile([P, n * D], fp32, tag="hp")
                    for k in range(C):
                        nc.tensor.matmul(
                            out=hp[:],
                            lhsT=ident[:],
                            rhs=ub[g][:, k * n * D : (k + 1) * n * D],
                            start=(k == 0),
                            stop=(k == C - 1),
                        )
                    # PSUM f32 -> SBUF bf16 on the Scalar engine
                    nc.scalar.activation(
                        out=h_sb[:, c0 * D : (c0 + n) * D], in_=hp[:],
                        func=COPY,
                    )
                    if g + 1 < NPE:
                        pe_filler(18)
                else:
                    # tail group: binary-tree h-sum on DVE over the k-major
                    # bf16 slabs, last fold writes straight into h_sb
                    u5 = ub[g]
                    nc.vector.tensor_add(
                        out=u5[:, 0 : 4 * n * D], in0=u5[:, 0 : 4 * n * D],
                        in1=u5[:, 4 * n * D : 8 * n * D],
                    )
                    nc.vector.tensor_add(
                        out=u5[:, 0 : 2 * n * D], in0=u5[:, 0 : 2 * n * D],
                        in1=u5[:, 2 * n * D : 4 * n * D],
                    )
                    nc.vector.tensor_add(
                        out=h_sb[:, c0 * D : (c0 + n) * D],
                        in0=u5[:, 0 : n * D], in1=u5[:, n * D : 2 * n * D],
                    )

                # m[p, c, j, d] = w[p, c, j, d] * h[p, c, d], in place over w
                w4 = wbt[g].rearrange("p (c j d) -> p c j d", c=n, j=J)
                h4 = h16[:, c0 : c0 + n]
                nc.vector.tensor_mul(
                    out=w4,
                    in0=w4,
                    in1=h4[:, :, None, :].broadcast_to([P, n, J, D]),
                )
                # fold the innermost 128 -> 16 with bf16 adds before the
                # (1x-rate) TensorReduce
                for w_ in (64, 32, 16):
                    nc.vector.tensor_add(
                        out=w4[:, :, :, 0:w_],
                        in0=w4[:, :, :, 0:w_],
                        in1=w4[:, :, :, w_ : 2 * w_],
                    )
                # raw dots (f32); j=0 was sign-folded in the multiply
                sc = sc_pool.tile([P, n * J], fp32, tag="sc")
                sc3 = sc[:].rearrange("p (c j) -> p c j", j=J)
                nc.vector.tensor_reduce(
                    out=sc3, in_=w4[:, :, :, 0:16], axis=X, op=ADD,
                )
                # softplus(x) = ln(1 + exp(x)); accumulate into lp[:, g]
                sp = sp_pool.tile([P, n * J], fp32, tag="sp")
                nc.scalar.activation(out=sp[:], in_=sc[:], func=EXP,
                                     scale=ACT_SCALE)
                nc.scalar.activation(out=sp[:], in_=sp[:], func=LN, bias=1.0,
                                     accum_out=lp[:, g : g + 1])

            # per-partition loss, then collapse partitions via ones-matmul
            lp1 = fin_pool.tile([P, 1], fp32, tag="lp1")
            nc.vector.tensor_reduce(out=lp1[:], in_=lp[:], axis=X, op=ADD)
            acc = ps_pool.tile([1, 1], fp32)
            nc.tensor.matmul(out=acc[:], lhsT=ones[:], rhs=lp1[:],
                             start=True, stop=True)
            out_sb = fin_pool.tile([1, 1], fp32, tag="out")
            nc.vector.tensor_copy(out=out_sb[:], in_=acc[:])
            nc.sync.dma_start(out=loss[:], in_=out_sb[:])

    nc.compile()
    return nc


def _get_nc():
    if "nc" not in _NC_CACHE:
        _NC_CACHE["nc"] = _build_bass()
    return _NC_CACHE["nc"]


def _make_in_maps(pos_u, pos_w, neg_w, u_emb, w_emb):
    pos_u = np.asarray(pos_u).astype(np.int32)
    pos_w = np.asarray(pos_w).astype(np.int32)
    neg_w = np.asarray(neg_w).astype(np.int32)
    u_emb = np.asarray(u_emb, dtype=np.float32)
    w_emb = np.asarray(w_emb, dtype=np.float32)

    fp8_np = mybir.dt.np(mybir.dt.float8e4)
    w_s = np.clip(w_emb * S_W, -224.0, 224.0)
    emb_cat = np.ascontiguousarray(
        np.concatenate(
            [np.clip(u_emb * S_U, -224.0, 224.0), w_s, -w_s], axis=0
        ).astype(fp8_np)
    )

    in_maps = []
    for i in range(N_CORES):
        sl = slice(i * B_LOC, (i + 1) * B_LOC)
        # batch row b -> (chunk c = b // 128, partition p = b % 128)
        rows_u = pos_u[sl].reshape(N_CHUNK, P, C)          # [c, p, k]
        rows_w = np.concatenate(
            [pos_w[sl, None] + 2 * V, neg_w[sl] + V], axis=1
        ).reshape(N_CHUNK, P, J)                           # [c, p, j]
        # columns: [u0k .. u4k | w0 .. w4 | u5k w5]; u blocks k-major
        cols = []
        for g in range(NPE):
            c0, n = STARTS[g], GROUPS[g]
            cols.append(rows_u[c0 : c0 + n].transpose(1, 2, 0).reshape(P, -1))
        for g in range(NPE):
            c0, n = STARTS[g], GROUPS[g]
            cols.append(rows_w[c0 : c0 + n].transpose(1, 0, 2).reshape(P, -1))
        c0, n = STARTS[NPE], GROUPS[NPE]
        cols.append(rows_u[c0 : c0 + n].transpose(1, 2, 0).reshape(P, -1))
        cols.append(rows_w[c0 : c0 + n].transpose(1, 0, 2).reshape(P, -1))
        gidx = np.concatenate(cols, axis=1)                # [P, N_CHUNK * R]
        in_maps.append(
            {
                "emb_cat": emb_cat,
                "gidx": np.ascontiguousarray(gidx),
            }
        )
    return in_maps


def _install_axon_profile_shim():
    """Provide antenv.axon_hooks (missing in this image) so trace=True can
    capture NTFF profiles via the axon PJRT .so, and keep trace artifacts
    local instead of uploading to a bucket."""
    import contextlib
    import ctypes
    import types

    import concourse.bass_utils as bu

    bu.upload_artifacts = lambda tmpdir: tmpdir

    try:
        from antenv.axon_hooks import get_axon_ntff_profile_hook  # noqa: F401

        return
    except ImportError:
        pass

    mod = types.ModuleType("antenv.axon_hooks")
    holder = {}
    mod.set_axon_ntff_profile_hook = lambda h: holder.__setitem__("h", h)
    mod.get_axon_ntff_profile_hook = lambda: holder.get("h")
    sys.modules["antenv.axon_hooks"] = mod
    import antenv

    antenv.axon_hooks = mod

    so_path = "/opt/axon/libaxon_pjrt.so"
    lib = ctypes.CDLL(so_path)
    if not hasattr(lib, "axon_start_nrt_profile"):
        return
    lib.axon_start_nrt_profile.argtypes = [
        ctypes.POINTER(ctypes.c_int64),
        ctypes.c_size_t,
    ]
    lib.axon_start_nrt_profile.restype = ctypes.c_int64
    lib.axon_stop_nrt_profile.argtypes = [ctypes.c_char_p]
    lib.axon_stop_nrt_profile.restype = ctypes.c_int64

    @contextlib.contextmanager
    def _hook(output_dir, device_ids):
        import jax

        jax.devices()
        if device_ids:
            ids = (ctypes.c_int64 * len(device_ids))(*device_ids)
            rc = lib.axon_start_nrt_profile(ids, len(device_ids))
        else:
            rc = lib.axon_start_nrt_profile(None, 0)
        if rc != 0:
            raise RuntimeError(f"axon_start_nrt_profile rc={rc}")
        try:
            yield
        finally:
            n = lib.axon_stop_nrt_profile(str(output_dir).encode())
            print(f"profile: {n} file(s) written to {output_dir}")

    mod.set_axon_ntff_profile_hook(_hook)


def _run(in_maps, trace=False):
    if trace:
        _install_axon_profile_shim()
    nc = _get_nc()
    return run_bass_kernel_spmd(nc, in_maps, list(range(N_CORES)), trace=trace)


def kernel(pos_u, pos_w, neg_w, u_emb, w_emb):
    in_maps = _make_in_maps(pos_u, pos_w, neg_w, u_emb, w_emb)
    bkr = _run(in_maps, trace=False)
    total = 0.0
    for r in bkr.results:
        total += float(r["loss_part"].astype(np.float64).sum())
    return np.float32(total)


def kernel_traced(pos_u, pos_w, neg_w, u_emb, w_emb):
    """Like kernel() but returns (loss, BassKernelResults) with HW profile."""
    in_maps = _make_in_maps(pos_u, pos_w, neg_w, u_emb, w_emb)
    bkr = _run(in_maps, trace=True)
    total = 0.0
    for r in bkr.results:
        total += float(r["loss_part"].astype(np.float64).sum())
    return np.float32(total), bkr


# revision 11
# speedup vs baseline: 1.4238x; 1.0107x over previous
"""CBOW negative-sampling loss on 8 Trainium2 NeuronCores.

Strategy: replicate the embedding tables, data-parallel over the batch dim.
Each core handles 2048 of the 16384 batch rows.

Host side: u_emb (x1024), w_emb (x32) and -w_emb are concatenated into one
[3V, D] fp8-e4m3 table (neg_w indices offset by +V, pos_w by +2V into the
negated copy, so every score product already carries its loss sign and the
dot reduce needs no sign handling). The gather traffic is descriptor-rate-
bound (~10ns/row/engine), so the layout splits u and w gathers:
  - u-rows are gathered raw fp8 (no cast) and consumed by the TensorEngine,
    which handles fp8 natively;
  - w-rows are gathered with an on-the-fly SWDGE cast to bf16 so the DVE
    keeps its 2x 16-bit rate for the score products.
The scale factors keep fp8 values in the normal range and are divided back
out in the activation's scale factor.

Per-core kernel layout:
  - batch row b -> chunk c = b // 128, partition p = b % 128.
  - 16 chunks in ramped groups. Per group one u-gather (k-major slabs: all
    k=0 rows, then k=1, ...) and one w-gather; w0+w1 share one gather and
    the last group uses a single combined cast gather. Transfer order is
    tuned so the first w block lands right after u0 (DVE starts early).
  - h = sum of the 8 context embeddings: 8 accumulating identity-weight
    matmuls per group (rhs = the k-th fp8 slab) on the TensorEngine ->
    PSUM f32, then one ACT copy PSUM -> SBUF bf16. The last (1-chunk)
    group sums on DVE instead to shorten the tail chain.
  - dots on DVE: one broadcast-mult (2x bf16 rate), 3 contiguous fold-adds
    128->16, one TensorReduce -> softplus(+x) terms for every column.
  - the TensorEngine's HAM activity window is kept hot with filler matmuls
    so the h-sum runs at 2.4 GHz instead of the cold 1.2 GHz.
  - softplus per group on ACT: Exp then Ln(1+x) with accum_out -> lp[:, g].
    Both functions come from one activation table (id 6) preloaded at
    kernel start, so no mid-kernel table swaps.
  - finale: reduce lp rows, PE ones-matmul to collapse partitions -> [1,1],
    host sums the 8 per-core scalars.

loss = sum_b softplus(-score_b) + sum_{b,k} softplus(+neg_score_bk)
"""

import sys

import numpy as np

sys.path.insert(0, "/opt/trn_rl_repo")

from concourse import bacc, bass, mybir, tile  # noqa: E402
from concourse.bass_utils import run_bass_kernel_spmd  # noqa: E402
from concourse.masks import make_identity  # noqa: E402

V, D = 100000, 128
B, C, K = 16384, 8, 5
N_CORES = 8
P = 128
B_LOC = B // N_CORES            # 2048 batch rows per core
N_CHUNK = B_LOC // P            # 16 chunks of 128 rows
GROUPS = (1, 3, 4, 4, 3, 1)     # chunks per gather group; last is combined
assert sum(GROUPS) == N_CHUNK
# matmul moving operand and a PSUM bank both cap at 512 f32 per partition
assert max(GROUPS) * D <= 512
J = 1 + K                       # 6 w-rows per batch row (pos + negs)
R = C + J                       # 14 gathered rows per batch row
NG = len(GROUPS)
NPE = NG - 1                    # groups whose h-sum runs on the PE
STARTS = [sum(GROUPS[:g]) for g in range(NG)]
# gidx column layout: [u0k .. u4k | w0 .. w4 | u5k w5]
U_OFF = [sum(C * n for n in GROUPS[:g]) for g in range(NPE)]
W_BASE = sum(C * n for n in GROUPS[:NPE])
W_OFF = [W_BASE + sum(J * n for n in GROUPS[:g]) for g in range(NPE)]
G5_OFF = W_BASE + sum(J * n for n in GROUPS[:NPE])
assert G5_OFF + R * GROUPS[NPE] == N_CHUNK * R

S_U = 1024.0                    # host-side scale into fp8 normal range
S_W = 32.0
ACT_SCALE = 1.0 / (S_U * S_W)   # divided back out inside the Exp

_NC_CACHE = {}


def _build_bass():
    nc = bacc.Bacc(
        "TRN2",
        target_bir_lowering=False,
        debug=False,
        dynamic_dma_scratch_size=65536,
    )

    bf16 = mybir.dt.bfloat16
    fp32 = mybir.dt.float32
    fp8 = mybir.dt.float8e4
    X = mybir.AxisListType.X
    ADD = mybir.AluOpType.add
    MUL = mybir.AluOpType.mult
    EXP = mybir.ActivationFunctionType.Exp
    LN = mybir.ActivationFunctionType.Ln
    COPY = mybir.ActivationFunctionType.Copy

    emb = nc.dram_tensor("emb_cat", [3 * V, D], fp8, kind="ExternalInput")
    gidx = nc.dram_tensor(
        "gidx", [P, N_CHUNK * R], mybir.dt.int32, kind="ExternalInput"
    )
    loss = nc.dram_tensor("loss_part", [1, 1], fp32, kind="ExternalOutput")

    with tile.TileContext(nc) as tc:
        with (
            tc.tile_pool(name="idx", bufs=1) as idx_pool,
            tc.tile_pool(name="ub", bufs=5) as ub_pool,
            tc.tile_pool(name="wb", bufs=5) as wb_pool,
            tc.tile_pool(name="sc", bufs=2) as sc_pool,
            tc.tile_pool(name="sp", bufs=2) as sp_pool,
            tc.tile_pool(name="fin", bufs=1) as fin_pool,
            tc.tile_pool(name="hp", bufs=3, space="PSUM") as hp_pool,
            tc.tile_pool(name="ps", bufs=1, space="PSUM") as ps_pool,
        ):
            # one table (id 6: natural_log_exp_and_others) serves Exp AND Ln
            nc.scalar.add_instruction(mybir.InstLoadActFuncSet(
                name=nc.get_next_instruction_name(), act_func_set_id=6,
                ins=[], outs=[]))

            ix_all = idx_pool.tile([P, N_CHUNK * R], mybir.dt.int32, tag="ix")
            n0 = C * GROUPS[0]   # u0 columns only: first desc-gen starts ASAP
            nc.sync.dma_start(out=ix_all[:, 0:n0], in_=gidx[:, 0:n0])
            nc.sync.dma_start(out=ix_all[:, n0:], in_=gidx[:, n0:])

            ident = fin_pool.tile([P, P], fp8, tag="ident")
            make_identity(nc, ident[:])
            ones = fin_pool.tile([P, 1], fp32, tag="ones")
            nc.gpsimd.memset(ones[:], 1.0)

            warm = ps_pool.tile([P, P], fp32, tag="warm")

            def pe_filler(count):
                # keep the TensorEngine's activity window hot; results unused
                for _ in range(count):
                    nc.tensor.matmul(out=warm[:], lhsT=ident[:],
                                     rhs=ident[:], start=True, stop=True)

            pe_filler(45)

            h_sb = fin_pool.tile([P, N_CHUNK * D], bf16, tag="h_sb")
            h16 = h_sb[:].rearrange("p (c d) -> p c d", c=N_CHUNK)
            lp = fin_pool.tile([P, NG], fp32, tag="lp")

            def gather(cols, ncols, dtype, tag, pool):
                t = pool.tile([P, ncols * D], dtype, tag=tag)
                nc.gpsimd.indirect_dma_start(
                    out=t[:],
                    out_offset=None,
                    in_=emb[:],
                    in_offset=bass.IndirectOffsetOnAxis(
                        ap=ix_all[:, cols : cols + ncols], axis=0
                    ),
                )
                return t

            # issue all gathers up front; the list order is the transfer
            # order: u0 | w0 | u5+w5 | u1 | w1 | u2 | w2 | u3 | w3 | u4 | w4
            # (the tail group's data comes early: its DVE work is emitted
            # right after g0's to fill the pipeline-fill bubble)
            ub, wbt = {}, {}
            ub[0] = gather(U_OFF[0], C * GROUPS[0], fp8, "ub", ub_pool)
            wbt[0] = gather(W_OFF[0], J * GROUPS[0], bf16, "wb", wb_pool)[:]
            g5 = gather(G5_OFF, R * GROUPS[NPE], bf16, "g5", wb_pool)
            ub[NPE] = g5[:, 0 : C * GROUPS[NPE] * D]
            wbt[NPE] = g5[:, C * GROUPS[NPE] * D :]
            ub[1] = gather(U_OFF[1], C * GROUPS[1], fp8, "ub", ub_pool)
            wbt[1] = gather(W_OFF[1], J * GROUPS[1], bf16, "wb", wb_pool)[:]
            ub[2] = gather(U_OFF[2], C * GROUPS[2], fp8, "ub", ub_pool)
            wbt[2] = gather(W_OFF[2], J * GROUPS[2], bf16, "wb", wb_pool)[:]
            ub[3] = gather(U_OFF[3], C * GROUPS[3], fp8, "ub", ub_pool)
            wbt[3] = gather(W_OFF[3], J * GROUPS[3], bf16, "wb", wb_pool)[:]
            ub[4] = gather(U_OFF[4], C * GROUPS[4], fp8, "ub", ub_pool)
            wbt[4] = gather(W_OFF[4], J * GROUPS[4], bf16, "wb", wb_pool)[:]

            for g in [0, NG - 1] + list(range(1, NPE)):
                n = GROUPS[g]
                c0 = STARTS[g]
                if g < NPE:
                    # h-sum on the TensorEngine: 8 accumulating identity
                    # matmuls (rhs = the k-th fp8 u-slab) -> PSUM f32
                    hp = hp_pool.tile([P, n * D], fp32, tag="hp")
                    for k in range(C):
                        nc.tensor.matmul(
                            out=hp[:],
                            lhsT=ident[:],
                            rhs=ub[g][:, k * n * D : (k + 1) * n * D],
                            start=(k == 0),
                            stop=(k == C - 1),
                        )
                    # PSUM f32 -> SBUF bf16 on the Scalar engine
                    nc.scalar.activation(
                        out=h_sb[:, c0 * D : (c0 + n) * D], in_=hp[:],
                        func=COPY,
                    )
                    if g + 1 < NPE:
                        pe_filler(30)
                else:
                    # tail group: binary-tree h-sum on DVE over the k-major
                    # bf16 slabs, last fold writes straight into h_sb
                    u5 = ub[g]
                    nc.vector.tensor_add(
                        out=u5[:, 0 : 4 * n * D], in0=u5[:, 0 : 4 * n * D],
                        in1=u5[:, 4 * n * D : 8 * n * D],
                    )
                    nc.vector.tensor_add(
                        out=u5[:, 0 : 2 * n * D], in0=u5[:, 0 : 2 * n * D],
                        in1=u5[:, 2 * n * D : 4 * n * D],
                    )
                    nc.vector.tensor_add(
                        out=h_sb[:, c0 * D : (c0 + n) * D],
                        in0=u5[:, 0 : n * D], in1=u5[:, n * D : 2 * n * D],
                    )

                # m[p, c, j, d] = w[p, c, j, d] * h[p, c, d], in place over w
                w4 = wbt[g].rearrange("p (c j d) -> p c j d", c=n, j=J)
                h4 = h16[:, c0 : c0 + n]
                nc.vector.tensor_mul(
                    out=w4,
                    in0=w4,
                    in1=h4[:, :, None, :].broadcast_to([P, n, J, D]),
                )
                # fold the innermost 128 -> 16 with bf16 adds before the
                # (1x-rate) TensorReduce
                for w_ in (64, 32, 16):
                    nc.vector.tensor_add(
                        out=w4[:, :, :, 0:w_],
                        in0=w4[:, :, :, 0:w_],
                        in1=w4[:, :, :, w_ : 2 * w_],
                    )
                # raw dots (f32); j=0 was sign-folded in the multiply
                sc = sc_pool.tile([P, n * J], fp32, tag="sc")
                sc3 = sc[:].rearrange("p (c j) -> p c j", j=J)
                nc.vector.tensor_reduce(
                    out=sc3, in_=w4[:, :, :, 0:16], axis=X, op=ADD,
                )
                # softplus(x) = ln(1 + exp(x)); accumulate into lp[:, g]
                sp = sp_pool.tile([P, n * J], fp32, tag="sp")
                nc.scalar.activation(out=sp[:], in_=sc[:], func=EXP,
                                     scale=ACT_SCALE)
                nc.scalar.activation(out=sp[:], in_=sp[:], func=LN, bias=1.0,
                                     accum_out=lp[:, g : g + 1])

            # per-partition loss, then collapse partitions via ones-matmul
            lp1 = fin_pool.tile([P, 1], fp32, tag="lp1")
            nc.vector.tensor_reduce(out=lp1[:], in_=lp[:], axis=X, op=ADD)
            acc = ps_pool.tile([1, 1], fp32)
            nc.tensor.matmul(out=acc[:], lhsT=ones[:], rhs=lp1[:],
                             start=True, stop=True)
            out_sb = fin_pool.tile([1, 1], fp32, tag="out")
            nc.vector.tensor_copy(out=out_sb[:], in_=acc[:])
            nc.sync.dma_start(out=loss[:], in_=out_sb[:])

    nc.compile()
    return nc


def _get_nc():
    if "nc" not in _NC_CACHE:
        _NC_CACHE["nc"] = _build_bass()
    return _NC_CACHE["nc"]


def _make_in_maps(pos_u, pos_w, neg_w, u_emb, w_emb):
    pos_u = np.asarray(pos_u).astype(np.int32)
    pos_w = np.asarray(pos_w).astype(np.int32)
    neg_w = np.asarray(neg_w).astype(np.int32)
    u_emb = np.asarray(u_emb, dtype=np.float32)
    w_emb = np.asarray(w_emb, dtype=np.float32)

    fp8_np = mybir.dt.np(mybir.dt.float8e4)
    w_s = np.clip(w_emb * S_W, -224.0, 224.0)
    emb_cat = np.ascontiguousarray(
        np.concatenate(
            [np.clip(u_emb * S_U, -224.0, 224.0), w_s, -w_s], axis=0
        ).astype(fp8_np)
    )

    in_maps = []
    for i in range(N_CORES):
        sl = slice(i * B_LOC, (i + 1) * B_LOC)
        # batch row b -> (chunk c = b // 128, partition p = b % 128)
        rows_u = pos_u[sl].reshape(N_CHUNK, P, C)          # [c, p, k]
        rows_w = np.concatenate(
            [pos_w[sl, None] + 2 * V, neg_w[sl] + V], axis=1
        ).reshape(N_CHUNK, P, J)                           # [c, p, j]
        # columns: [u0k .. u4k | w0 .. w4 | u5k w5]; u blocks k-major
        cols = []
        for g in range(NPE):
            c0, n = STARTS[g], GROUPS[g]
            cols.append(rows_u[c0 : c0 + n].transpose(1, 2, 0).reshape(P, -1))
        for g in range(NPE):
            c0, n = STARTS[g], GROUPS[g]
            cols.append(rows_w[c0 : c0 + n].transpose(1, 0, 2).reshape(P, -1))
        c0, n = STARTS[NPE], GROUPS[NPE]
        cols.append(rows_u[c0 : c0 + n].transpose(1, 2, 0).reshape(P, -1))
        cols.append(rows_w[c0 : c0 + n].transpose(1, 0, 2).reshape(P, -1))
        gidx = np.concatenate(cols, axis=1)                # [P, N_CHUNK * R]
        in_maps.append(
            {
                "emb_cat": emb_cat,
                "gidx": np.ascontiguousarray(gidx),
            }
        )
    return in_maps


def _install_axon_profile_shim():
    """Provide antenv.axon_hooks (missing in this image) so trace=True can
    capture NTFF profiles via the axon PJRT .so, and keep trace artifacts
    local instead of uploading to a bucket."""
    import contextlib
    import ctypes
    import types

    import concourse.bass_utils as bu

    bu.upload_artifacts = lambda tmpdir: tmpdir

    try:
        from antenv.axon_hooks import get_axon_ntff_profile_hook  # noqa: F401

        return
    except ImportError:
        pass

    mod = types.ModuleType("antenv.axon_hooks")
    holder = {}
    mod.set_axon_ntff_profile_hook = lambda h: holder.__setitem__("h", h)
    mod.get_axon_ntff_profile_hook = lambda: holder.get("h")
    sys.modules["antenv.axon_hooks"] = mod
    import antenv

    antenv.axon_hooks = mod

    so_path = "/opt/axon/libaxon_pjrt.so"
    lib = ctypes.CDLL(so_path)
    if not hasattr(lib, "axon_start_nrt_profile"):
        return
    lib.axon_start_nrt_profile.argtypes = [
        ctypes.POINTER(ctypes.c_int64),
        ctypes.c_size_t,
    ]
    lib.axon_start_nrt_profile.restype = ctypes.c_int64
    lib.axon_stop_nrt_profile.argtypes = [ctypes.c_char_p]
    lib.axon_stop_nrt_profile.restype = ctypes.c_int64

    @contextlib.contextmanager
    def _hook(output_dir, device_ids):
        import jax

        jax.devices()
        if device_ids:
            ids = (ctypes.c_int64 * len(device_ids))(*device_ids)
            rc = lib.axon_start_nrt_profile(ids, len(device_ids))
        else:
            rc = lib.axon_start_nrt_profile(None, 0)
        if rc != 0:
            raise RuntimeError(f"axon_start_nrt_profile rc={rc}")
        try:
            yield
        finally:
            n = lib.axon_stop_nrt_profile(str(output_dir).encode())
            print(f"profile: {n} file(s) written to {output_dir}")

    mod.set_axon_ntff_profile_hook(_hook)


def _run(in_maps, trace=False):
    if trace:
        _install_axon_profile_shim()
    nc = _get_nc()
    return run_bass_kernel_spmd(nc, in_maps, list(range(N_CORES)), trace=trace)


def kernel(pos_u, pos_w, neg_w, u_emb, w_emb):
    in_maps = _make_in_maps(pos_u, pos_w, neg_w, u_emb, w_emb)
    bkr = _run(in_maps, trace=False)
    total = 0.0
    for r in bkr.results:
        total += float(r["loss_part"].astype(np.float64).sum())
    return np.float32(total)


def kernel_traced(pos_u, pos_w, neg_w, u_emb, w_emb):
    """Like kernel() but returns (loss, BassKernelResults) with HW profile."""
    in_maps = _make_in_maps(pos_u, pos_w, neg_w, u_emb, w_emb)
    bkr = _run(in_maps, trace=True)
    total = 0.0
    for r in bkr.results:
        total += float(r["loss_part"].astype(np.float64).sum())
    return np.float32(total), bkr
